# revision 33
# baseline (speedup 1.0000x reference)
"""Masked multi-head attention (sparse_attention) on 8 trn2 NeuronCores.

Sharding: query rows are split 8 ways (512 rows per core); every core
computes all 8 heads for its rows, so each core reads only its
[512, 4096] slice of the adjacency mask.

Device algorithm per core (scores kept transposed, [keys, queries]):
  qT = Wq^T @ xT[:, my_cols]          [256, 512]  (+bq via scalar-add)
  kT = Wk^T @ xT                      [256, 4096] (+bk via scalar-add)
  V' = [x @ Wv | 1 | zeros] per head  [4096, 8*64] fp16 (ones col at 32)
  for head group G (heads 4G..4G+3), k-tile t (128 keys):
      S^T[t] = K_h Q_h^T  (4 heads row-tiled on PE, K=32) -> PSUM
      E = exp(S^T / sqrt(32))         (one ACT op per head pair)
      P = E * adjT[t]                 (DVE fp16 tensor_tensor, 2x mode)
      bankA += [O_h0; r_h0 | O_h1; r_h1] = V'^T @ P  (col-tiled M=64)
      bankB += same for heads h2, h3
  per bank: broadcast r via Sel matmul, reciprocal, scale O strips;
  final^T = Wo4^T @ oTb (host-permuted zero-padded Wo) + bo'
  where bo' = bv @ Wo + bo (bv folded out of V on host).
  Output written transposed [256, 512]; host transposes back.

Projections are interleaved into the attention loop so the ACT engine
(exp throughput, the bottleneck at ~133us) starts almost immediately.
"""

import contextlib
import sys

import numpy as np

for _p in ("/opt/trn_rl_repo",):
    if _p not in sys.path:
        sys.path.insert(0, _p)

import concourse.bass as bass
import concourse.mybir as mybir
import concourse.tile as tile
from concourse import bacc, bass_utils

F16 = mybir.dt.float16
F32 = mybir.dt.float32
F32R = mybir.dt.float32r
AF = mybir.ActivationFunctionType
ALU = mybir.AluOpType

N = 4096
D = 256
H = 8
HD = 32
NCORES = 8
NQ = N // NCORES  # 512 queries per core
KT = N // 128  # 32 key tiles
SCALE = float(1.0 / np.sqrt(np.float32(HD)))
W64 = 64  # per-head stationary block: [V(32) | ones | zeros(31)]


def build_kernel(nc: bass.Bass, repeat: int = 1):
    xT_d = nc.dram_tensor("xT", [D, N], F16, kind="ExternalInput").ap()
    xq_d = nc.dram_tensor("xq", [D, NQ], F16, kind="ExternalInput").ap()
    adjb_d = nc.dram_tensor("adjb", [NQ, N], F16, kind="ExternalInput").ap()
    Wq_d = nc.dram_tensor("Wq", [D, D], F16, kind="ExternalInput").ap()
    Wk_d = nc.dram_tensor("Wk", [D, D], F16, kind="ExternalInput").ap()
    Wv_d = nc.dram_tensor("Wv", [D, D], F16, kind="ExternalInput").ap()
    Wo4_d = nc.dram_tensor("Wo4", [512, D], F32R, kind="ExternalInput").ap()
    sel_d = nc.dram_tensor("sel", [128, 128], F32, kind="ExternalInput").ap()
    bq2_d = nc.dram_tensor("bq2", [128, 2], F32, kind="ExternalInput").ap()
    bk2_d = nc.dram_tensor("bk2", [128, 2], F32, kind="ExternalInput").ap()
    bo2_d = nc.dram_tensor("bo2", [128, 2], F32, kind="ExternalInput").ap()
    out_d = nc.dram_tensor("out", [D, NQ], F32, kind="ExternalOutput").ap()

    with tile.TileContext(nc, num_cores=NCORES) as tc:
        for _ in range(repeat):
            with contextlib.ExitStack() as ctx:
                build_body(ctx, tc, xT_d, xq_d, adjb_d, Wq_d, Wk_d, Wv_d,
                           Wo4_d, sel_d, bq2_d, bk2_d, bo2_d, out_d)
    return nc


def build_body(ctx, tc, xT_d, xq_d, adjb_d, Wq_d, Wk_d, Wv_d, Wo4_d,
               sel_d, bq2_d, bk2_d, bo2_d, out_d):
    nc = tc.nc
    const = ctx.enter_context(tc.tile_pool(name="const", bufs=1))
    big = ctx.enter_context(tc.tile_pool(name="big", bufs=1))
    ppool = ctx.enter_context(tc.tile_pool(name="p", bufs=3))
    psum_pv = ctx.enter_context(tc.tile_pool(name="pspv", bufs=1, space="PSUM"))
    psum_qk = ctx.enter_context(tc.tile_pool(name="psqk", bufs=2, space="PSUM"))
    psum_pj = ctx.enter_context(tc.tile_pool(name="pspj", bufs=2, space="PSUM"))

    # ---- SBUF tiles ----
    Wq_s = const.tile([128, 2, D], F16, tag="wq")
    Wk_s = const.tile([128, 2, D], F16, tag="wk")
    Wv_s = const.tile([128, 2, D], F16, tag="wv")
    Wo4_s = const.tile([128, 4, D], F32R, tag="wo4")
    sel_s = const.tile([128, 128], F32, tag="sel")
    bq2_s = const.tile([128, 2], F32, tag="bq2")
    bk2_s = const.tile([128, 2], F32, tag="bk2")
    bo2_s = const.tile([128, 2], F32, tag="bo2")

    xT_s = big.tile([128, 2, N], F16, tag="xT")
    xq_s = big.tile([128, 2, NQ], F16, tag="xq")
    kT_s = big.tile([128, 2, N], F32R, tag="kT")
    qT_s = big.tile([128, 2, NQ], F32R, tag="qT")
    V_s = big.tile([128, KT, H * W64], F16, tag="V")
    adjT_s = big.tile([128, KT * NQ], F16, tag="adjT")
    v4 = V_s[:].rearrange("p t (h w) -> p t h w", w=W64)

    # ---- DMA queue in critical-path order (batched: each SP-queue
    # entry costs ~600ns dispatch, so combine 2-half transfers) ----
    # First QK needs Wq+xq (Q proj) and Wk+xT[j=0] (K proj chunk 0).
    nc.sync.dma_start(Wq_s[:], Wq_d.rearrange("(c p) d -> p c d", c=2))
    nc.sync.dma_start(xq_s[:], xq_d.rearrange("(c p) n -> p c n", c=2))
    nc.sync.dma_start(Wk_s[:], Wk_d.rearrange("(c p) d -> p c d", c=2))
    nc.sync.dma_start(
        xT_s[:, :, 0:512],
        xT_d[:, 0:512].rearrange("(c p) n -> p c n", c=2))
    nc.sync.dma_start(bq2_s[:], bq2_d)
    nc.sync.dma_start(bk2_s[:], bk2_d)
    nc.sync.dma_start(Wv_s[:], Wv_d.rearrange("(c p) d -> p c d", c=2))
    # adj^T via xbar DMA transpose: [512,128] -> [128,512] per k-tile,
    # interleaved with the remaining xT column chunks
    for t in range(4):
        nc.sync.dma_start_transpose(
            out=adjT_s[:, t * NQ:(t + 1) * NQ],
            in_=adjb_d[:, t * 128:(t + 1) * 128])
    for j in range(1, 8):
        nc.sync.dma_start(
            xT_s[:, :, j * 512:(j + 1) * 512],
            xT_d[:, j * 512:(j + 1) * 512].rearrange(
                "(c p) n -> p c n", c=2))
        for t in range(4 * j, 4 * j + 4):
            nc.sync.dma_start_transpose(
                out=adjT_s[:, t * NQ:(t + 1) * NQ],
                in_=adjb_d[:, t * 128:(t + 1) * 128])
    nc.sync.dma_start(sel_s[:], sel_d)
    nc.sync.dma_start(bo2_s[:], bo2_d)
    nc.sync.dma_start(Wo4_s[:], Wo4_d.rearrange("(b p) d -> p b d", b=4))

    # ---- projection emitters ----
    for m in range(2):
        pt = psum_pj.tile([128, 512], F32, tag="pj")
        for c in range(2):
            nc.tensor.matmul(
                pt[:], Wq_s[:, c, m * 128:(m + 1) * 128], xq_s[:, c],
                start=(c == 0), stop=(c == 1))
        nc.vector.tensor_scalar_add(qT_s[:, m], pt[:], bq2_s[:, m:m + 1])

    def emit_kproj(G, j, eng=None):
        pt = psum_pj.tile([128, 512], F32, tag="pj")
        for c in range(2):
            nc.tensor.matmul(
                pt[:], Wk_s[:, c, G * 128:(G + 1) * 128],
                xT_s[:, c, j * 512:(j + 1) * 512],
                start=(c == 0), stop=(c == 1))
        (eng or nc.vector).tensor_scalar_add(
            kT_s[:, G, j * 512:(j + 1) * 512], pt[:], bk2_s[:, G:G + 1])

    def emit_vproj(t):
        # V' tail: ones column then zero pad (Pool engine, idle anyway)
        nc.gpsimd.memset(v4[:, t, :, HD:HD + 1], 1.0)
        nc.gpsimd.memset(v4[:, t, :, HD + 1:], 0.0)
        pt = psum_pj.tile([128, 512], F32, tag="pj")
        for c in range(2):
            nc.tensor.matmul(
                pt[:, :D], xT_s[:, c, t * 128:(t + 1) * 128], Wv_s[:, c],
                start=(c == 0), stop=(c == 1))
        nc.vector.tensor_copy(
            v4[:, t, :, 0:HD],
            pt[:, :D].rearrange("p (h w) -> p h w", w=HD))

    emit_kproj(0, 0)
    emit_kproj(0, 1)
    emit_vproj(0)
    emit_vproj(1)

    # ---- attention ----
    # oTb bank layout: tile b in {0: G0 bankA, 1: G0 bankB, 2: G1 A, 3: G1 B}
    # per bank: partitions 0:32 = O_h(even), 32 = r_h(even), 33:64 zeros,
    #           64:96 = O_h(odd), 96 = r_h(odd), 97:128 zeros
    oTb = big.tile([128, 4, NQ], F32R, tag="oTb")
    fT = big.tile([128, 2, NQ], F32, tag="fT")

    def emit_gtail(G, banks):
        # r rows live at bank partitions 32 (even head) and 96 (odd).
        # Copy them into a ones-filled tile on the idle Pool engine, then one
        # f32r sel-matmul per bank broadcasts each row across its
        # 64-partition half; recip+scale on DVE. Phased emission so the
        # PE sel matmuls never wait on the backlogged DVE queue.
        rsbs = []
        for bk in range(2):
            rsb = ppool.tile([128, NQ], F32, tag="rsb")
            nc.gpsimd.memset(rsb[:], 1.0)
            nc.vector.tensor_copy(rsb[32:33, :], banks[bk][32:33, :])
            nc.vector.tensor_copy(rsb[96:97, :], banks[bk][96:97, :])
            rsbs.append(rsb)
        rxs = []
        for bk in range(2):
            rx = psum_qk.tile([128, 2 * NQ], F32, tag="qk")
            nc.tensor.matmul(rx[:, :NQ], sel_s[:], rsbs[bk][:],
                             start=True, stop=True)
            rxs.append(rx)
        rrs = []
        for bk in range(2):
            rr = ppool.tile([128, NQ], F32, tag="rr")
            nc.vector.reciprocal(rr[:], rxs[bk][:, :NQ])
            rrs.append(rr)
        for bk in range(2):
            b = 2 * G + bk
            nc.vector.tensor_tensor(
                oTb[:, b], banks[bk][:], rrs[bk][:], op=ALU.mult)

    wo_pts = [None, None]

    prev_banks = None
    pending = []  # shared PV software-pipeline lag, drains across G
    for G in range(2):
        bankA = psum_pv.tile([128, NQ], F32, tag="opsA")
        bankB = psum_pv.tile([128, NQ], F32, tag="opsB")
        banks = [bankA, bankB]

        def emit_pv(P, t, p, banks=banks, G=G):
            for ii in range(2):
                h = 4 * G + 2 * P + ii
                nc.tensor.matmul(
                    banks[P][64 * ii:64 * (ii + 1), :],
                    V_s[:, t, W64 * h:W64 * (h + 1)],
                    p[:, ii * NQ:(ii + 1) * NQ],
                    start=(t == 0), stop=(t == KT - 1),
                    tile_position=(0, 64 * ii),
                    skip_group_check=True,
                )

        for t in range(KT):
            if G == 0:
                if t + 2 < KT:
                    emit_vproj(t + 2)
                if t % 4 == 2 and t // 4 + 2 < 8:
                    emit_kproj(0, t // 4 + 2)
                if t >= 16 and t % 2 == 0:
                    emit_kproj(1, (t - 16) // 2)
            else:
                if t == 3:
                    # Wo partial accumulation over G0's oTb banks;
                    # reuses the pj tag's two buffers (no kproj/vproj
                    # allocations happen after this point)
                    for m in range(2):
                        wo_pt = psum_pj.tile([128, 512], F32, tag="pj")
                        wo_pts[m] = wo_pt
                        for b in range(2):
                            nc.tensor.matmul(
                                wo_pts[m][:],
                                Wo4_s[:, b, m * 128:(m + 1) * 128],
                                oTb[:, b], start=(b == 0), stop=False,
                                skip_group_check=True)
            for P in range(2):  # head pair (4G+2P, 4G+2P+1) -> banks[P]
                if G == 1 and t == 1 and P == 1:
                    # G0's last PV drained from `pending` at (t=1, P=0)
                    # and G1's first PV (same PSUM banks, bufs=1) is
                    # emitted at (t=1, P=1): the only safe window to
                    # read G0's banks.
                    emit_gtail(0, prev_banks)
                qk = psum_qk.tile([128, 2 * NQ], F32, tag="qk")
                for ii in range(2):
                    i = 2 * P + ii
                    nc.tensor.matmul(
                        qk[:, ii * NQ:(ii + 1) * NQ],
                        kT_s[32 * i:32 * (i + 1), G, t * 128:(t + 1) * 128],
                        qT_s[32 * i:32 * (i + 1), G, :],
                        start=True, stop=True,
                        tile_position=(32 * i, 0),
                    )
                e = ppool.tile([128, 2 * NQ], F16, tag="e")
                nc.scalar.activation(e[:], qk[:], AF.Exp, bias=0.0,
                                     scale=SCALE)
                p = ppool.tile([128, 2 * NQ], F16, tag="p")
                for ii in range(2):
                    nc.vector.tensor_tensor(
                        p[:, ii * NQ:(ii + 1) * NQ],
                        e[:, ii * NQ:(ii + 1) * NQ],
                        adjT_s[:, t * NQ:(t + 1) * NQ],
                        op=ALU.mult)
                pending.append((emit_pv, P, t, p))
                if len(pending) > 2:
                    fn, *args = pending.pop(0)
                    fn(*args)
        prev_banks = banks

    for fn, *args in pending:
        fn(*args)
    emit_gtail(1, prev_banks)

    # ---- finish output projection (transposed; host untransposes) ----
    for m in range(2):
        for b in range(2, 4):
            nc.tensor.matmul(wo_pts[m][:],
                             Wo4_s[:, b, m * 128:(m + 1) * 128],
                             oTb[:, b], start=False, stop=(b == 3),
                             skip_group_check=True)
        nc.vector.tensor_scalar_add(fT[:, m], wo_pts[m][:],
                                    bo2_s[:, m:m + 1])
        nc.sync.dma_start(out_d[m * 128:(m + 1) * 128, :], fT[:, m])


_CACHED = {}


def _get_built(repeat: int = 1):
    key = ("nc", repeat)
    if key not in _CACHED:
        nc = bacc.Bacc("TRN2", target_bir_lowering=False, debug=False,
                       num_devices=NCORES)
        build_kernel(nc, repeat=repeat)
        nc.finalize()
        _CACHED[key] = nc
    return _CACHED[key]


def prep_in_maps(x, adj, Wq, bq, Wk, bk, Wv, bv, Wo, bo):
    x = np.asarray(x, np.float32)
    adj = np.asarray(adj, np.float32)

    xT = np.ascontiguousarray(x[0].T).astype(np.float16)  # [256, 4096]
    adjb = adj[0].astype(np.float16)
    bq2 = np.ascontiguousarray(np.asarray(bq, np.float32).reshape(2, 128).T)
    bk2 = np.ascontiguousarray(np.asarray(bk, np.float32).reshape(2, 128).T)

    Wo = np.asarray(Wo, np.float32)
    # bv folded out of V': out = (P @ V)/r + bv, so bo' = bv @ Wo + bo
    bo_eff = (np.asarray(bv, np.float32) @ Wo
              + np.asarray(bo, np.float32)).astype(np.float32)
    bo2 = np.ascontiguousarray(bo_eff.reshape(2, 128).T)

    Wo4 = np.zeros((4, 128, D), np.float32)
    for b in range(4):
        G, isB = divmod(b, 2)
        for hh in range(2):
            h = 4 * G + 2 * isB + hh
            Wo4[b, 64 * hh:64 * hh + 32, :] = Wo[32 * h:32 * h + 32, :]
    Wo4 = np.ascontiguousarray(Wo4.reshape(512, D))


    sel = np.zeros((128, 128), np.float32)
    sel[32, 0:64] = 1.0
    sel[96, 64:128] = 1.0

    common = dict(
        xT=xT,
        Wq=np.ascontiguousarray(np.asarray(Wq, np.float32)).astype(np.float16),
        Wk=np.ascontiguousarray(np.asarray(Wk, np.float32)).astype(np.float16),
        Wv=np.ascontiguousarray(np.asarray(Wv, np.float32)).astype(np.float16),
        Wo4=Wo4, sel=sel,
        bq2=bq2, bk2=bk2, bo2=bo2,
    )
    in_maps = []
    for c in range(NCORES):
        m = dict(common)
        m["xq"] = np.ascontiguousarray(xT[:, c * NQ:(c + 1) * NQ])
        m["adjb"] = np.ascontiguousarray(adjb[c * NQ:(c + 1) * NQ, :])
        in_maps.append(m)
    return in_maps


def kernel(x, adj, Wq, bq, Wk, bk, Wv, bv, Wo, bo, trace=False):
    nc = _get_built()
    in_maps = prep_in_maps(x, adj, Wq, bq, Wk, bk, Wv, bv, Wo, bo)

    res = bass_utils.run_bass_kernel_spmd(
        nc, in_maps, core_ids=list(range(NCORES)), trace=trace)
    out = np.concatenate(
        [np.ascontiguousarray(r["out"].T) for r in res.results], axis=0)
    kernel.last_results = res
    return out[None, :, :].astype(np.float32)


# revision 34
# speedup vs baseline: 2.2999x; 2.2999x over previous
"""Masked multi-head attention (sparse_attention) on 8 trn2 NeuronCores.

Sharding: query rows are split 8 ways (512 rows per core); every core
computes all 8 heads for its rows, so each core reads only its
[512, 4096] slice of the adjacency mask.

Device algorithm per core (scores kept transposed, [keys, queries]):
  qT = Wq^T @ xT[:, my_cols]          [256, 512]  (+bq via scalar-add)
  kT = Wk^T @ xT                      [256, 4096] (+bk via scalar-add)
  V' = [x @ Wv | 1 | zeros] per head  [4096, 8*64] fp16 (ones col at 32)
  for head group G (heads 4G..4G+3), k-tile t (128 keys):
      S^T[t] = K_h Q_h^T  (4 heads row-tiled on PE, K=32) -> PSUM
      E = exp(S^T / sqrt(32))         (one ACT op per head pair)
      P = E * adjT[t]                 (DVE fp16 tensor_tensor, 2x mode)
      bankA += [O_h0; r_h0 | O_h1; r_h1] = V'^T @ P  (col-tiled M=64)
      bankB += same for heads h2, h3
  per bank: broadcast r via Sel matmul, reciprocal, scale O strips;
  final^T = Wo4^T @ oTb (host-permuted zero-padded Wo) + bo'
  where bo' = bv @ Wo + bo (bv folded out of V on host).
  Output written transposed [256, 512]; host transposes back.

Projections are interleaved into the attention loop so the ACT engine
(exp throughput, the bottleneck at ~133us) starts almost immediately.
"""

import contextlib
import sys

import numpy as np

for _p in ("/opt/trn_rl_repo",):
    if _p not in sys.path:
        sys.path.insert(0, _p)

import concourse.bass as bass
import concourse.mybir as mybir
import concourse.tile as tile
from concourse import bacc, bass_utils

F16 = mybir.dt.float16
F32 = mybir.dt.float32
F32R = mybir.dt.float32r
AF = mybir.ActivationFunctionType
ALU = mybir.AluOpType

N = 4096
D = 256
H = 8
HD = 32
NCORES = 8
NQ = N // NCORES  # 512 queries per core
KT = N // 128  # 32 key tiles
SCALE = float(1.0 / np.sqrt(np.float32(HD)))
W64 = 64  # per-head stationary block: [V(32) | ones | zeros(31)]


def build_kernel(nc: bass.Bass, repeat: int = 1):
    xT_d = nc.dram_tensor("xT", [D, N], F16, kind="ExternalInput").ap()
    xq_d = nc.dram_tensor("xq", [D, NQ], F16, kind="ExternalInput").ap()
    adjb_d = nc.dram_tensor("adjb", [NQ, N], F16, kind="ExternalInput").ap()
    Wq_d = nc.dram_tensor("Wq", [D, D], F16, kind="ExternalInput").ap()
    Wk_d = nc.dram_tensor("Wk", [D, D], F16, kind="ExternalInput").ap()
    Wv_d = nc.dram_tensor("Wv", [D, D], F16, kind="ExternalInput").ap()
    Wo4_d = nc.dram_tensor("Wo4", [512, D], F32R, kind="ExternalInput").ap()
    sel_d = nc.dram_tensor("sel", [128, 128], F32, kind="ExternalInput").ap()
    bq2_d = nc.dram_tensor("bq2", [128, 2], F32, kind="ExternalInput").ap()
    bk2_d = nc.dram_tensor("bk2", [128, 2], F32, kind="ExternalInput").ap()
    bo2_d = nc.dram_tensor("bo2", [128, 2], F32, kind="ExternalInput").ap()
    out_d = nc.dram_tensor("out", [D, NQ], F32, kind="ExternalOutput").ap()

    with tile.TileContext(nc, num_cores=NCORES) as tc:
        for _ in range(repeat):
            with contextlib.ExitStack() as ctx:
                build_body(ctx, tc, xT_d, xq_d, adjb_d, Wq_d, Wk_d, Wv_d,
                           Wo4_d, sel_d, bq2_d, bk2_d, bo2_d, out_d)
    return nc


def build_body(ctx, tc, xT_d, xq_d, adjb_d, Wq_d, Wk_d, Wv_d, Wo4_d,
               sel_d, bq2_d, bk2_d, bo2_d, out_d):
    nc = tc.nc
    const = ctx.enter_context(tc.tile_pool(name="const", bufs=1))
    big = ctx.enter_context(tc.tile_pool(name="big", bufs=1))
    ppool = ctx.enter_context(tc.tile_pool(name="p", bufs=3))
    psum_pv = ctx.enter_context(tc.tile_pool(name="pspv", bufs=1, space="PSUM"))
    psum_qk = ctx.enter_context(tc.tile_pool(name="psqk", bufs=2, space="PSUM"))
    psum_pj = ctx.enter_context(tc.tile_pool(name="pspj", bufs=2, space="PSUM"))

    # ---- SBUF tiles ----
    Wq_s = const.tile([128, 2, D], F16, tag="wq")
    Wk_s = const.tile([128, 2, D], F16, tag="wk")
    Wv_s = const.tile([128, 2, D], F16, tag="wv")
    Wo4_s = const.tile([128, 4, D], F32R, tag="wo4")
    sel_s = const.tile([128, 128], F32, tag="sel")
    bq2_s = const.tile([128, 2], F32, tag="bq2")
    bk2_s = const.tile([128, 2], F32, tag="bk2")
    bo2_s = const.tile([128, 2], F32, tag="bo2")

    xT_s = big.tile([128, 2, N], F16, tag="xT")
    xq_s = big.tile([128, 2, NQ], F16, tag="xq")
    kT_s = big.tile([128, 2, N], F32R, tag="kT")
    qT_s = big.tile([128, 2, NQ], F32R, tag="qT")
    V_s = big.tile([128, KT, H * W64], F16, tag="V")
    adjT_s = big.tile([128, KT * NQ], F16, tag="adjT")
    v4 = V_s[:].rearrange("p t (h w) -> p t h w", w=W64)

    # ---- DMA queue in critical-path order (batched: each SP-queue
    # entry costs ~600ns dispatch, so combine 2-half transfers) ----
    # First QK needs Wq+xq (Q proj) and Wk+xT[j=0] (K proj chunk 0).
    nc.sync.dma_start(Wq_s[:], Wq_d.rearrange("(c p) d -> p c d", c=2))
    nc.sync.dma_start(xq_s[:], xq_d.rearrange("(c p) n -> p c n", c=2))
    nc.sync.dma_start(Wk_s[:], Wk_d.rearrange("(c p) d -> p c d", c=2))
    nc.sync.dma_start(
        xT_s[:, :, 0:512],
        xT_d[:, 0:512].rearrange("(c p) n -> p c n", c=2))
    nc.sync.dma_start(bq2_s[:], bq2_d)
    nc.sync.dma_start(bk2_s[:], bk2_d)
    nc.sync.dma_start(Wv_s[:], Wv_d.rearrange("(c p) d -> p c d", c=2))
    # adj^T via xbar DMA transpose: [512,128] -> [128,512] per k-tile,
    # interleaved with the remaining xT column chunks
    for t in range(4):
        nc.sync.dma_start_transpose(
            out=adjT_s[:, t * NQ:(t + 1) * NQ],
            in_=adjb_d[:, t * 128:(t + 1) * 128])
    for j in range(1, 8):
        nc.sync.dma_start(
            xT_s[:, :, j * 512:(j + 1) * 512],
            xT_d[:, j * 512:(j + 1) * 512].rearrange(
                "(c p) n -> p c n", c=2))
        for t in range(4 * j, 4 * j + 4):
            nc.sync.dma_start_transpose(
                out=adjT_s[:, t * NQ:(t + 1) * NQ],
                in_=adjb_d[:, t * 128:(t + 1) * 128])
    nc.sync.dma_start(sel_s[:], sel_d)
    nc.sync.dma_start(bo2_s[:], bo2_d)
    nc.sync.dma_start(Wo4_s[:], Wo4_d.rearrange("(b p) d -> p b d", b=4))

    # ---- projection emitters ----
    for m in range(2):
        pt = psum_pj.tile([128, 512], F32, tag="pj")
        for c in range(2):
            nc.tensor.matmul(
                pt[:], Wq_s[:, c, m * 128:(m + 1) * 128], xq_s[:, c],
                start=(c == 0), stop=(c == 1))
        nc.vector.tensor_scalar_add(qT_s[:, m], pt[:], bq2_s[:, m:m + 1])

    def emit_kproj(G, j, eng=None):
        pt = psum_pj.tile([128, 512], F32, tag="pj")
        for c in range(2):
            nc.tensor.matmul(
                pt[:], Wk_s[:, c, G * 128:(G + 1) * 128],
                xT_s[:, c, j * 512:(j + 1) * 512],
                start=(c == 0), stop=(c == 1))
        (eng or nc.vector).tensor_scalar_add(
            kT_s[:, G, j * 512:(j + 1) * 512], pt[:], bk2_s[:, G:G + 1])

    def emit_vproj(t):
        # V' tail: ones column then zero pad (Pool engine, idle anyway)
        nc.gpsimd.memset(v4[:, t, :, HD:HD + 1], 1.0)
        nc.gpsimd.memset(v4[:, t, :, HD + 1:], 0.0)
        pt = psum_pj.tile([128, 512], F32, tag="pj")
        for c in range(2):
            nc.tensor.matmul(
                pt[:, :D], xT_s[:, c, t * 128:(t + 1) * 128], Wv_s[:, c],
                start=(c == 0), stop=(c == 1))
        nc.vector.tensor_copy(
            v4[:, t, :, 0:HD],
            pt[:, :D].rearrange("p (h w) -> p h w", w=HD))

    emit_kproj(0, 0)
    emit_kproj(0, 1)
    emit_vproj(0)
    emit_vproj(1)

    # ---- attention ----
    # oTb bank layout: tile b in {0: G0 bankA, 1: G0 bankB, 2: G1 A, 3: G1 B}
    # per bank: partitions 0:32 = O_h(even), 32 = r_h(even), 33:64 zeros,
    #           64:96 = O_h(odd), 96 = r_h(odd), 97:128 zeros
    oTb = big.tile([128, 4, NQ], F32R, tag="oTb")
    fT = big.tile([128, 2, NQ], F32, tag="fT")

    def emit_gtail(G, banks):
        # r rows live at bank partitions 32 (even head) and 96 (odd).
        # Copy them into a ones-filled tile on the idle Pool engine, then one
        # f32r sel-matmul per bank broadcasts each row across its
        # 64-partition half; recip+scale on DVE. Phased emission so the
        # PE sel matmuls never wait on the backlogged DVE queue.
        rsbs = []
        for bk in range(2):
            rsb = ppool.tile([128, NQ], F32, tag="rsb")
            nc.gpsimd.memset(rsb[:], 1.0)
            nc.vector.tensor_copy(rsb[32:33, :], banks[bk][32:33, :])
            nc.vector.tensor_copy(rsb[96:97, :], banks[bk][96:97, :])
            rsbs.append(rsb)
        rxs = []
        for bk in range(2):
            rx = psum_qk.tile([128, 2 * NQ], F32, tag="qk")
            nc.tensor.matmul(rx[:, :NQ], sel_s[:], rsbs[bk][:],
                             start=True, stop=True)
            rxs.append(rx)
        rrs = []
        for bk in range(2):
            rr = ppool.tile([128, NQ], F32, tag="rr")
            nc.vector.reciprocal(rr[:], rxs[bk][:, :NQ])
            rrs.append(rr)
        for bk in range(2):
            b = 2 * G + bk
            nc.vector.tensor_tensor(
                oTb[:, b], banks[bk][:], rrs[bk][:], op=ALU.mult)

    wo_pts = [None, None]

    prev_banks = None
    pending = []  # shared PV software-pipeline lag, drains across G
    for G in range(2):
        bankA = psum_pv.tile([128, NQ], F32, tag="opsA")
        bankB = psum_pv.tile([128, NQ], F32, tag="opsB")
        banks = [bankA, bankB]

        def emit_pv(P, t, p, banks=banks, G=G):
            for ii in range(2):
                h = 4 * G + 2 * P + ii
                nc.tensor.matmul(
                    banks[P][64 * ii:64 * (ii + 1), :],
                    V_s[:, t, W64 * h:W64 * (h + 1)],
                    p[:, ii * NQ:(ii + 1) * NQ],
                    start=(t == 0), stop=(t == KT - 1),
                    tile_position=(0, 64 * ii),
                    skip_group_check=True,
                )

        for t in range(KT):
            if G == 0:
                if t + 2 < KT:
                    emit_vproj(t + 2)
                if t % 4 == 2 and t // 4 + 2 < 8:
                    emit_kproj(0, t // 4 + 2)
                if t >= 16 and t % 2 == 0:
                    emit_kproj(1, (t - 16) // 2)
            else:
                if t == 3:
                    # Wo partial accumulation over G0's oTb banks;
                    # reuses the pj tag's two buffers (no kproj/vproj
                    # allocations happen after this point)
                    for m in range(2):
                        wo_pt = psum_pj.tile([128, 512], F32, tag="pj")
                        wo_pts[m] = wo_pt
                        for b in range(2):
                            nc.tensor.matmul(
                                wo_pts[m][:],
                                Wo4_s[:, b, m * 128:(m + 1) * 128],
                                oTb[:, b], start=(b == 0), stop=False,
                                skip_group_check=True)
            for P in range(2):  # head pair (4G+2P, 4G+2P+1) -> banks[P]
                if G == 1 and t == 1 and P == 1:
                    # G0's last PV drained from `pending` at (t=1, P=0)
                    # and G1's first PV (same PSUM banks, bufs=1) is
                    # emitted at (t=1, P=1): the only safe window to
                    # read G0's banks.
                    emit_gtail(0, prev_banks)
                qk = psum_qk.tile([128, 2 * NQ], F32, tag="qk")
                for ii in range(2):
                    i = 2 * P + ii
                    nc.tensor.matmul(
                        qk[:, ii * NQ:(ii + 1) * NQ],
                        kT_s[32 * i:32 * (i + 1), G, t * 128:(t + 1) * 128],
                        qT_s[32 * i:32 * (i + 1), G, :],
                        start=True, stop=True,
                        tile_position=(32 * i, 0),
                    )
                e = ppool.tile([128, 2 * NQ], F16, tag="e")
                nc.scalar.activation(e[:], qk[:], AF.Exp, bias=0.0,
                                     scale=SCALE)
                p = ppool.tile([128, 2 * NQ], F16, tag="p")
                for ii in range(2):
                    nc.vector.tensor_tensor(
                        p[:, ii * NQ:(ii + 1) * NQ],
                        e[:, ii * NQ:(ii + 1) * NQ],
                        adjT_s[:, t * NQ:(t + 1) * NQ],
                        op=ALU.mult)
                pending.append((emit_pv, P, t, p))
                if len(pending) > 2:
                    fn, *args = pending.pop(0)
                    fn(*args)
        prev_banks = banks

    for fn, *args in pending:
        fn(*args)
    emit_gtail(1, prev_banks)

    # ---- finish output projection (transposed; host untransposes) ----
    for m in range(2):
        for b in range(2, 4):
            nc.tensor.matmul(wo_pts[m][:],
                             Wo4_s[:, b, m * 128:(m + 1) * 128],
                             oTb[:, b], start=False, stop=(b == 3),
                             skip_group_check=True)
        nc.vector.tensor_scalar_add(fT[:, m], wo_pts[m][:],
                                    bo2_s[:, m:m + 1])
        nc.sync.dma_start(out_d[m * 128:(m + 1) * 128, :], fT[:, m])


_CACHED = {}


def _get_built(repeat: int = 1):
    key = ("nc", repeat)
    if key not in _CACHED:
        nc = bacc.Bacc("TRN2", target_bir_lowering=False, debug=False,
                       num_devices=NCORES)
        build_kernel(nc, repeat=repeat)
        nc.finalize()
        _CACHED[key] = nc
    return _CACHED[key]


def prep_in_maps(x, adj, Wq, bq, Wk, bk, Wv, bv, Wo, bo):
    x = np.asarray(x, np.float32)
    adj = np.asarray(adj, np.float32)

    xT = np.ascontiguousarray(x[0].T).astype(np.float16)  # [256, 4096]
    adjb = adj[0].astype(np.float16)
    bq2 = np.ascontiguousarray(np.asarray(bq, np.float32).reshape(2, 128).T)
    bk2 = np.ascontiguousarray(np.asarray(bk, np.float32).reshape(2, 128).T)

    Wo = np.asarray(Wo, np.float32)
    # bv folded out of V': out = (P @ V)/r + bv, so bo' = bv @ Wo + bo
    bo_eff = (np.asarray(bv, np.float32) @ Wo
              + np.asarray(bo, np.float32)).astype(np.float32)
    bo2 = np.ascontiguousarray(bo_eff.reshape(2, 128).T)

    Wo4 = np.zeros((4, 128, D), np.float32)
    for b in range(4):
        G, isB = divmod(b, 2)
        for hh in range(2):
            h = 4 * G + 2 * isB + hh
            Wo4[b, 64 * hh:64 * hh + 32, :] = Wo[32 * h:32 * h + 32, :]
    Wo4 = np.ascontiguousarray(Wo4.reshape(512, D))


    sel = np.zeros((128, 128), np.float32)
    sel[32, 0:64] = 1.0
    sel[96, 64:128] = 1.0

    common = dict(
        xT=xT,
        Wq=np.ascontiguousarray(np.asarray(Wq, np.float32)).astype(np.float16),
        Wk=np.ascontiguousarray(np.asarray(Wk, np.float32)).astype(np.float16),
        Wv=np.ascontiguousarray(np.asarray(Wv, np.float32)).astype(np.float16),
        Wo4=Wo4, sel=sel,
        bq2=bq2, bk2=bk2, bo2=bo2,
    )
    in_maps = []
    for c in range(NCORES):
        m = dict(common)
        m["xq"] = np.ascontiguousarray(xT[:, c * NQ:(c + 1) * NQ])
        m["adjb"] = np.ascontiguousarray(adjb[c * NQ:(c + 1) * NQ, :])
        in_maps.append(m)
    return in_maps


def _get_runner():
    """Build the jitted SPMD executable once and cache it across calls
    (a fresh jax.jit per call costs ~1.25s of retrace + XLA compile)."""
    if "runner" in _CACHED:
        return _CACHED["runner"]
    import jax
    from jax.sharding import Mesh, PartitionSpec
    from jax.experimental.shard_map import shard_map
    from concourse.bass2jax import (
        _bass_exec_p, partition_id_tensor, install_neuronx_cc_hook)

    install_neuronx_cc_hook()
    nc = _get_built()
    partition_name = (nc.partition_id_tensor.name
                      if nc.partition_id_tensor else None)
    in_names, out_names, out_avals = [], [], []
    for alloc in nc.m.functions[0].allocations:
        if not isinstance(alloc, mybir.MemoryLocationSet):
            continue
        name = alloc.memorylocations[0].name
        if alloc.kind == "ExternalInput":
            if name != partition_name:
                in_names.append(name)
        elif alloc.kind == "ExternalOutput":
            out_names.append(name)
            out_avals.append(jax.core.ShapedArray(
                tuple(alloc.tensor_shape), mybir.dt.np(alloc.dtype)))
    n_params = len(in_names)
    in_names_all = list(in_names) + list(out_names)
    if partition_name:
        in_names_all.append(partition_name)

    def _body(*args):
        operands = list(args)
        if partition_name is not None:
            operands.append(partition_id_tensor())
        return tuple(_bass_exec_p.bind(
            *operands, out_avals=tuple(out_avals),
            in_names=tuple(in_names_all), out_names=tuple(out_names),
            lowering_input_output_aliases=(), sim_require_finite=True,
            sim_require_nnan=True, nc=nc))

    devices = jax.devices()[:NCORES]
    mesh = Mesh(np.asarray(devices), ("core",))
    nin = n_params + len(out_names)
    fn = jax.jit(
        shard_map(_body, mesh=mesh,
                  in_specs=(PartitionSpec("core"),) * nin,
                  out_specs=(PartitionSpec("core"),) * len(out_names),
                  check_rep=False),
        keep_unused=True)
    _CACHED["runner"] = (fn, in_names, out_names, out_avals)
    return _CACHED["runner"]


def kernel(x, adj, Wq, bq, Wk, bk, Wv, bv, Wo, bo, trace=False):
    in_maps = prep_in_maps(x, adj, Wq, bq, Wk, bk, Wv, bv, Wo, bo)
    fn, in_names, out_names, out_avals, = _get_runner()
    per_core = [[np.asarray(m[name]) for name in in_names] for m in in_maps]
    concat_in = [
        np.concatenate([per_core[c][i] for c in range(NCORES)], axis=0)
        for i in range(len(in_names))
    ]
    concat_zeros = [
        np.zeros((NCORES * a.shape[0], *a.shape[1:]), a.dtype)
        for a in out_avals
    ]
    outs = fn(*concat_in, *concat_zeros)
    oi = out_names.index("out")
    res = np.asarray(outs[oi]).reshape(NCORES, D, NQ)
    out = np.concatenate([res[c].T for c in range(NCORES)], axis=0)
    kernel.last_results = None
    return out[None, :, :].astype(np.float32)


# revision 35
# speedup vs baseline: 3.0462x; 1.3245x over previous
"""Masked multi-head attention (sparse_attention) on 8 trn2 NeuronCores.

Sharding: query rows are split 8 ways (512 rows per core); every core
computes all 8 heads for its rows, so each core reads only its
[512, 4096] slice of the adjacency mask.

Device algorithm per core (scores kept transposed, [keys, queries]):
  qT = Wq^T @ xT[:, my_cols]          [256, 512]  (+bq via scalar-add)
  kT = Wk^T @ xT                      [256, 4096] (+bk via scalar-add)
  V' = [x @ Wv | 1 | zeros] per head  [4096, 8*64] fp16 (ones col at 32)
  for head group G (heads 4G..4G+3), k-tile t (128 keys):
      S^T[t] = K_h Q_h^T  (4 heads row-tiled on PE, K=32) -> PSUM
      E = exp(S^T / sqrt(32))         (one ACT op per head pair)
      P = E * adjT[t]                 (DVE fp16 tensor_tensor, 2x mode)
      bankA += [O_h0; r_h0 | O_h1; r_h1] = V'^T @ P  (col-tiled M=64)
      bankB += same for heads h2, h3
  per bank: broadcast r via Sel matmul, reciprocal, scale O strips;
  final^T = Wo4^T @ oTb (host-permuted zero-padded Wo) + bo'
  where bo' = bv @ Wo + bo (bv folded out of V on host).
  Output written transposed [256, 512]; host transposes back.

Projections are interleaved into the attention loop so the ACT engine
(exp throughput, the bottleneck at ~133us) starts almost immediately.
"""

import contextlib
import sys

import numpy as np

for _p in ("/opt/trn_rl_repo",):
    if _p not in sys.path:
        sys.path.insert(0, _p)

import concourse.bass as bass
import concourse.mybir as mybir
import concourse.tile as tile
from concourse import bacc, bass_utils

F16 = mybir.dt.float16
F32 = mybir.dt.float32
F32R = mybir.dt.float32r
AF = mybir.ActivationFunctionType
ALU = mybir.AluOpType

N = 4096
D = 256
H = 8
HD = 32
NCORES = 8
NQ = N // NCORES  # 512 queries per core
KT = N // 128  # 32 key tiles
SCALE = float(1.0 / np.sqrt(np.float32(HD)))
W64 = 64  # per-head stationary block: [V(32) | ones | zeros(31)]


def build_kernel(nc: bass.Bass, repeat: int = 1):
    xT_d = nc.dram_tensor("xT", [D, N], F16, kind="ExternalInput").ap()
    xq_d = nc.dram_tensor("xq", [D, NQ], F16, kind="ExternalInput").ap()
    adjb_d = nc.dram_tensor("adjb", [NQ, N], F16, kind="ExternalInput").ap()
    Wq_d = nc.dram_tensor("Wq", [D, D], F16, kind="ExternalInput").ap()
    Wk_d = nc.dram_tensor("Wk", [D, D], F16, kind="ExternalInput").ap()
    Wv_d = nc.dram_tensor("Wv", [D, D], F16, kind="ExternalInput").ap()
    Wo4_d = nc.dram_tensor("Wo4", [512, D], F32R, kind="ExternalInput").ap()
    sel_d = nc.dram_tensor("sel", [128, 128], F32, kind="ExternalInput").ap()
    bq2_d = nc.dram_tensor("bq2", [128, 2], F32, kind="ExternalInput").ap()
    bk2_d = nc.dram_tensor("bk2", [128, 2], F32, kind="ExternalInput").ap()
    bo2_d = nc.dram_tensor("bo2", [128, 2], F32, kind="ExternalInput").ap()
    out_d = nc.dram_tensor("out", [D, NQ], F32, kind="ExternalOutput").ap()

    with tile.TileContext(nc, num_cores=NCORES) as tc:
        for _ in range(repeat):
            with contextlib.ExitStack() as ctx:
                build_body(ctx, tc, xT_d, xq_d, adjb_d, Wq_d, Wk_d, Wv_d,
                           Wo4_d, sel_d, bq2_d, bk2_d, bo2_d, out_d)
    return nc


def build_body(ctx, tc, xT_d, xq_d, adjb_d, Wq_d, Wk_d, Wv_d, Wo4_d,
               sel_d, bq2_d, bk2_d, bo2_d, out_d):
    nc = tc.nc
    const = ctx.enter_context(tc.tile_pool(name="const", bufs=1))
    big = ctx.enter_context(tc.tile_pool(name="big", bufs=1))
    ppool = ctx.enter_context(tc.tile_pool(name="p", bufs=6))
    psum_pv = ctx.enter_context(tc.tile_pool(name="pspv", bufs=1, space="PSUM"))
    psum_qk = ctx.enter_context(tc.tile_pool(name="psqk", bufs=2, space="PSUM"))
    psum_pj = ctx.enter_context(tc.tile_pool(name="pspj", bufs=2, space="PSUM"))

    # ---- SBUF tiles ----
    Wq_s = const.tile([128, 2, D], F16, tag="wq")
    Wk_s = const.tile([128, 2, D], F16, tag="wk")
    Wv_s = const.tile([128, 2, D], F16, tag="wv")
    Wo4_s = const.tile([128, 4, D], F32R, tag="wo4")
    sel_s = const.tile([128, 128], F32, tag="sel")
    bq2_s = const.tile([128, 2], F32, tag="bq2")
    bk2_s = const.tile([128, 2], F32, tag="bk2")
    bo2_s = const.tile([128, 2], F32, tag="bo2")

    xT_s = big.tile([128, 2, N], F16, tag="xT")
    xq_s = big.tile([128, 2, NQ], F16, tag="xq")
    kT_s = big.tile([128, 2, N], F32R, tag="kT")
    qT_s = big.tile([128, 2, NQ], F32R, tag="qT")
    V_s = big.tile([128, KT, H * W64], F16, tag="V")
    adjT_s = big.tile([128, KT * NQ], F16, tag="adjT")
    v4 = V_s[:].rearrange("p t (h w) -> p t h w", w=W64)

    # ---- DMA queue in critical-path order (batched: each SP-queue
    # entry costs ~600ns dispatch, so combine 2-half transfers) ----
    # First QK needs Wq+xq (Q proj) and Wk+xT[j=0] (K proj chunk 0).
    nc.sync.dma_start(Wq_s[:], Wq_d.rearrange("(c p) d -> p c d", c=2))
    nc.sync.dma_start(xq_s[:], xq_d.rearrange("(c p) n -> p c n", c=2))
    nc.sync.dma_start(Wk_s[:], Wk_d.rearrange("(c p) d -> p c d", c=2))
    nc.sync.dma_start(
        xT_s[:, :, 0:512],
        xT_d[:, 0:512].rearrange("(c p) n -> p c n", c=2))
    nc.sync.dma_start(bq2_s[:], bq2_d)
    nc.sync.dma_start(bk2_s[:], bk2_d)
    nc.sync.dma_start(Wv_s[:], Wv_d.rearrange("(c p) d -> p c d", c=2))
    # adj^T via xbar DMA transpose: [512,128] -> [128,512] per k-tile,
    # interleaved with the remaining xT column chunks
    for t in range(4):
        nc.sync.dma_start_transpose(
            out=adjT_s[:, t * NQ:(t + 1) * NQ],
            in_=adjb_d[:, t * 128:(t + 1) * 128])
    for j in range(1, 8):
        nc.sync.dma_start(
            xT_s[:, :, j * 512:(j + 1) * 512],
            xT_d[:, j * 512:(j + 1) * 512].rearrange(
                "(c p) n -> p c n", c=2))
        for t in range(4 * j, 4 * j + 4):
            nc.sync.dma_start_transpose(
                out=adjT_s[:, t * NQ:(t + 1) * NQ],
                in_=adjb_d[:, t * 128:(t + 1) * 128])
    nc.sync.dma_start(sel_s[:], sel_d)
    nc.sync.dma_start(bo2_s[:], bo2_d)
    nc.sync.dma_start(Wo4_s[:], Wo4_d.rearrange("(b p) d -> p b d", b=4))

    # ---- projection emitters ----
    for m in range(2):
        pt = psum_pj.tile([128, 512], F32, tag="pj")
        for c in range(2):
            nc.tensor.matmul(
                pt[:], Wq_s[:, c, m * 128:(m + 1) * 128], xq_s[:, c],
                start=(c == 0), stop=(c == 1))
        nc.vector.tensor_scalar_add(qT_s[:, m], pt[:], bq2_s[:, m:m + 1])

    def emit_kproj(G, j, eng=None):
        pt = psum_pj.tile([128, 512], F32, tag="pj")
        for c in range(2):
            nc.tensor.matmul(
                pt[:], Wk_s[:, c, G * 128:(G + 1) * 128],
                xT_s[:, c, j * 512:(j + 1) * 512],
                start=(c == 0), stop=(c == 1))
        (eng or nc.vector).tensor_scalar_add(
            kT_s[:, G, j * 512:(j + 1) * 512], pt[:], bk2_s[:, G:G + 1])

    def emit_vproj(t):
        # V' tail: ones column then zero pad (Pool engine, idle anyway)
        nc.gpsimd.memset(v4[:, t, :, HD:HD + 1], 1.0)
        nc.gpsimd.memset(v4[:, t, :, HD + 1:], 0.0)
        pt = psum_pj.tile([128, 512], F32, tag="pj")
        for c in range(2):
            nc.tensor.matmul(
                pt[:, :D], xT_s[:, c, t * 128:(t + 1) * 128], Wv_s[:, c],
                start=(c == 0), stop=(c == 1))
        nc.vector.tensor_copy(
            v4[:, t, :, 0:HD],
            pt[:, :D].rearrange("p (h w) -> p h w", w=HD))

    emit_kproj(0, 0)
    emit_kproj(0, 1)
    emit_vproj(0)
    emit_vproj(1)

    # ---- attention ----
    # oTb bank layout: tile b in {0: G0 bankA, 1: G0 bankB, 2: G1 A, 3: G1 B}
    # per bank: partitions 0:32 = O_h(even), 32 = r_h(even), 33:64 zeros,
    #           64:96 = O_h(odd), 96 = r_h(odd), 97:128 zeros
    oTb = big.tile([128, 4, NQ], F32R, tag="oTb")
    fT = big.tile([128, 2, NQ], F32, tag="fT")

    def emit_gtail(G, banks):
        # r rows live at bank partitions 32 (even head) and 96 (odd).
        # Copy them into a ones-filled tile on the idle Pool engine, then one
        # f32r sel-matmul per bank broadcasts each row across its
        # 64-partition half; recip+scale on DVE. Phased emission so the
        # PE sel matmuls never wait on the backlogged DVE queue.
        rsbs = []
        for bk in range(2):
            rsb = ppool.tile([128, NQ], F32, tag="rsb")
            nc.gpsimd.memset(rsb[:], 1.0)
            nc.vector.tensor_copy(rsb[32:33, :], banks[bk][32:33, :])
            nc.vector.tensor_copy(rsb[96:97, :], banks[bk][96:97, :])
            rsbs.append(rsb)
        rxs = []
        for bk in range(2):
            rx = psum_qk.tile([128, 2 * NQ], F32, tag="qk")
            nc.tensor.matmul(rx[:, :NQ], sel_s[:], rsbs[bk][:],
                             start=True, stop=True)
            rxs.append(rx)
        rrs = []
        for bk in range(2):
            rr = ppool.tile([128, NQ], F32, tag="rr")
            nc.vector.reciprocal(rr[:], rxs[bk][:, :NQ])
            rrs.append(rr)
        for bk in range(2):
            b = 2 * G + bk
            nc.vector.tensor_tensor(
                oTb[:, b], banks[bk][:], rrs[bk][:], op=ALU.mult)

    wo_pts = [None, None]

    prev_banks = None
    pending = []  # shared PV software-pipeline lag, drains across G
    for G in range(2):
        bankA = psum_pv.tile([128, NQ], F32, tag="opsA")
        bankB = psum_pv.tile([128, NQ], F32, tag="opsB")
        banks = [bankA, bankB]

        def emit_pv(P, t, p, banks=banks, G=G):
            for ii in range(2):
                h = 4 * G + 2 * P + ii
                nc.tensor.matmul(
                    banks[P][64 * ii:64 * (ii + 1), :],
                    V_s[:, t, W64 * h:W64 * (h + 1)],
                    p[:, ii * NQ:(ii + 1) * NQ],
                    start=(t == 0), stop=(t == KT - 1),
                    tile_position=(0, 64 * ii),
                    skip_group_check=True,
                )

        for t in range(KT):
            if G == 0:
                if t + 2 < KT:
                    emit_vproj(t + 2)
                if t % 4 == 2 and t // 4 + 2 < 8:
                    emit_kproj(0, t // 4 + 2)
                if t >= 16 and t % 2 == 0:
                    emit_kproj(1, (t - 16) // 2)
            else:
                if t == 3:
                    # Wo partial accumulation over G0's oTb banks;
                    # reuses the pj tag's two buffers (no kproj/vproj
                    # allocations happen after this point)
                    for m in range(2):
                        wo_pt = psum_pj.tile([128, 512], F32, tag="pj")
                        wo_pts[m] = wo_pt
                        for b in range(2):
                            nc.tensor.matmul(
                                wo_pts[m][:],
                                Wo4_s[:, b, m * 128:(m + 1) * 128],
                                oTb[:, b], start=(b == 0), stop=False,
                                skip_group_check=True)
            for P in range(2):  # head pair (4G+2P, 4G+2P+1) -> banks[P]
                if G == 1 and t == 2 and P == 1:
                    # G0's last PV drained from `pending` at (t=1, P=0)
                    # and G1's first PV (same PSUM banks, bufs=1) is
                    # emitted at (t=1, P=1): the only safe window to
                    # read G0's banks.
                    emit_gtail(0, prev_banks)
                qk = psum_qk.tile([128, 2 * NQ], F32, tag="qk")
                for ii in range(2):
                    i = 2 * P + ii
                    nc.tensor.matmul(
                        qk[:, ii * NQ:(ii + 1) * NQ],
                        kT_s[32 * i:32 * (i + 1), G, t * 128:(t + 1) * 128],
                        qT_s[32 * i:32 * (i + 1), G, :],
                        start=True, stop=True,
                        tile_position=(32 * i, 0),
                    )
                e = ppool.tile([128, 2 * NQ], F16, tag="e")
                nc.scalar.activation(e[:], qk[:], AF.Exp, bias=0.0,
                                     scale=SCALE)
                p = ppool.tile([128, 2 * NQ], F16, tag="p")
                for ii in range(2):
                    # one of the two SBUF-only mask mults per head pair
                    # runs on the otherwise-idle Pool engine
                    eng = nc.gpsimd if ii == 1 else nc.vector
                    eng.tensor_tensor(
                        p[:, ii * NQ:(ii + 1) * NQ],
                        e[:, ii * NQ:(ii + 1) * NQ],
                        adjT_s[:, t * NQ:(t + 1) * NQ],
                        op=ALU.mult)
                pending.append((emit_pv, P, t, p))
                if len(pending) > 4:
                    fn, *args = pending.pop(0)
                    fn(*args)
        prev_banks = banks

    for fn, *args in pending:
        fn(*args)
    emit_gtail(1, prev_banks)

    # ---- finish output projection (transposed; host untransposes) ----
    for m in range(2):
        for b in range(2, 4):
            nc.tensor.matmul(wo_pts[m][:],
                             Wo4_s[:, b, m * 128:(m + 1) * 128],
                             oTb[:, b], start=False, stop=(b == 3),
                             skip_group_check=True)
        nc.vector.tensor_scalar_add(fT[:, m], wo_pts[m][:],
                                    bo2_s[:, m:m + 1])
        nc.sync.dma_start(out_d[m * 128:(m + 1) * 128, :], fT[:, m])


_CACHED = {}


def _get_built(repeat: int = 1):
    key = ("nc", repeat)
    if key not in _CACHED:
        nc = bacc.Bacc("TRN2", target_bir_lowering=False, debug=False,
                       num_devices=NCORES)
        build_kernel(nc, repeat=repeat)
        nc.finalize()
        _CACHED[key] = nc
    return _CACHED[key]


def prep_in_maps(x, adj, Wq, bq, Wk, bk, Wv, bv, Wo, bo):
    x = np.asarray(x, np.float32)
    adj = np.asarray(adj, np.float32)

    xT = np.ascontiguousarray(x[0].T).astype(np.float16)  # [256, 4096]
    adjb = adj[0].astype(np.float16)
    bq2 = np.ascontiguousarray(np.asarray(bq, np.float32).reshape(2, 128).T)
    bk2 = np.ascontiguousarray(np.asarray(bk, np.float32).reshape(2, 128).T)

    Wo = np.asarray(Wo, np.float32)
    # bv folded out of V': out = (P @ V)/r + bv, so bo' = bv @ Wo + bo
    bo_eff = (np.asarray(bv, np.float32) @ Wo
              + np.asarray(bo, np.float32)).astype(np.float32)
    bo2 = np.ascontiguousarray(bo_eff.reshape(2, 128).T)

    Wo4 = np.zeros((4, 128, D), np.float32)
    for b in range(4):
        G, isB = divmod(b, 2)
        for hh in range(2):
            h = 4 * G + 2 * isB + hh
            Wo4[b, 64 * hh:64 * hh + 32, :] = Wo[32 * h:32 * h + 32, :]
    Wo4 = np.ascontiguousarray(Wo4.reshape(512, D))


    sel = np.zeros((128, 128), np.float32)
    sel[32, 0:64] = 1.0
    sel[96, 64:128] = 1.0

    common = dict(
        xT=xT,
        Wq=np.ascontiguousarray(np.asarray(Wq, np.float32)).astype(np.float16),
        Wk=np.ascontiguousarray(np.asarray(Wk, np.float32)).astype(np.float16),
        Wv=np.ascontiguousarray(np.asarray(Wv, np.float32)).astype(np.float16),
        Wo4=Wo4, sel=sel,
        bq2=bq2, bk2=bk2, bo2=bo2,
    )
    in_maps = []
    for c in range(NCORES):
        m = dict(common)
        m["xq"] = np.ascontiguousarray(xT[:, c * NQ:(c + 1) * NQ])
        m["adjb"] = np.ascontiguousarray(adjb[c * NQ:(c + 1) * NQ, :])
        in_maps.append(m)
    return in_maps


def _get_runner():
    """Build the jitted SPMD executable once and cache it across calls
    (a fresh jax.jit per call costs ~1.25s of retrace + XLA compile)."""
    if "runner" in _CACHED:
        return _CACHED["runner"]
    import jax
    from jax.sharding import Mesh, PartitionSpec
    from jax.experimental.shard_map import shard_map
    from concourse.bass2jax import (
        _bass_exec_p, partition_id_tensor, install_neuronx_cc_hook)

    install_neuronx_cc_hook()
    nc = _get_built()
    partition_name = (nc.partition_id_tensor.name
                      if nc.partition_id_tensor else None)
    in_names, out_names, out_avals = [], [], []
    for alloc in nc.m.functions[0].allocations:
        if not isinstance(alloc, mybir.MemoryLocationSet):
            continue
        name = alloc.memorylocations[0].name
        if alloc.kind == "ExternalInput":
            if name != partition_name:
                in_names.append(name)
        elif alloc.kind == "ExternalOutput":
            out_names.append(name)
            out_avals.append(jax.core.ShapedArray(
                tuple(alloc.tensor_shape), mybir.dt.np(alloc.dtype)))
    n_params = len(in_names)
    in_names_all = list(in_names) + list(out_names)
    if partition_name:
        in_names_all.append(partition_name)

    def _body(*args):
        operands = list(args)
        if partition_name is not None:
            operands.append(partition_id_tensor())
        return tuple(_bass_exec_p.bind(
            *operands, out_avals=tuple(out_avals),
            in_names=tuple(in_names_all), out_names=tuple(out_names),
            lowering_input_output_aliases=(), sim_require_finite=True,
            sim_require_nnan=True, nc=nc))

    devices = jax.devices()[:NCORES]
    mesh = Mesh(np.asarray(devices), ("core",))
    nin = n_params + len(out_names)
    fn = jax.jit(
        shard_map(_body, mesh=mesh,
                  in_specs=(PartitionSpec("core"),) * nin,
                  out_specs=(PartitionSpec("core"),) * len(out_names),
                  check_rep=False),
        keep_unused=True)
    _CACHED["runner"] = (fn, in_names, out_names, out_avals)
    return _CACHED["runner"]


def kernel(x, adj, Wq, bq, Wk, bk, Wv, bv, Wo, bo, trace=False):
    in_maps = prep_in_maps(x, adj, Wq, bq, Wk, bk, Wv, bv, Wo, bo)
    fn, in_names, out_names, out_avals, = _get_runner()
    per_core = [[np.asarray(m[name]) for name in in_names] for m in in_maps]
    concat_in = [
        np.concatenate([per_core[c][i] for c in range(NCORES)], axis=0)
        for i in range(len(in_names))
    ]
    concat_zeros = [
        np.zeros((NCORES * a.shape[0], *a.shape[1:]), a.dtype)
        for a in out_avals
    ]
    outs = fn(*concat_in, *concat_zeros)
    oi = out_names.index("out")
    res = np.asarray(outs[oi]).reshape(NCORES, D, NQ)
    out = np.concatenate([res[c].T for c in range(NCORES)], axis=0)
    kernel.last_results = None
    return out[None, :, :].astype(np.float32)
